# revision 3
# baseline (speedup 1.0000x reference)
"""Multi-head attention (2-axis RoPE) Trainium2 kernel, 8-core data parallel.

Problem (hardcoded): B=16, S=1024 (32x32 grid), E=256, H=8, D=32, fp32.
  qkv = x @ Wqkv + bqkv ; RoPE(q), RoPE(k) ; softmax(q k^T / sqrt(D)) @ v ; @ Wout + bout

Sharding: batch across 8 cores (2 batches/core). Each core runs the full
attention for its 2 batches; host scatters inputs / gathers outputs.

Device layout strategy (per core, T=2048 tokens):
  - host passes x pre-transposed (feature-major) with a ones row appended:
    xT_aug [257, 2048]; weights augmented with bias rows so all bias adds
    happen inside the matmuls (K=1 ones-row chunks).
  - q,k computed feature-major qkT [512, T]; RoPE applied as
    x*cosT + P(x*sinT) where P is a partition pair-swap done on the PE.
  - v computed token-major [T, 256] with a ones column per head (v_aug) so the
    attention-value matmul also produces the softmax denominator row.
  - scores computed transposed, scoresT [sk, sq], 4 heads packed into the
    128x128 PE array via 32-row bands (K=D=32).  exp on ScalarE (the
    bottleneck engine: 16.8M elements/core).
  - AV: out_augT [33, sq] = v_aug.T @ attnT, 2 heads col-packed; divide by the
    denominator row via a K=1 ones broadcast matmul + DVE multiply, landing
    attn_outT [256, T] feature-major in SBUF.
  - out projection consumes attn_outT as lhsT per token tile, producing the
    final [T, 256] token-major so the output DMA is contiguous.
"""

import math

import numpy as np

B, G, H, D, E = 16, 32, 8, 32, 256
S = G * G
NCORES = 8
B_LOC = B // NCORES
T = B_LOC * S  # tokens per core
SCALE = 1.0 / math.sqrt(D)

ATTN_BF16 = True  # exp output / v in bf16 (halves SBUF + DVE traffic)

_COMPILED = None

# test.py knobs (default off; harness path unaffected)
TRACE = False
TRACE_DIR = None
LAST_RESULTS = None


def _rope_tables():
    """cosT/sinT [128, T] matching reference._rope_cos_sin, feature-major.

    Row p multiplies feature d = p % 32 of every head.  Device computes
    rope(x) = x*cosT + P(x*sinT) with P the pair swap (p ^ 1), so the sign
    pattern sits pre-permutation: even rows +sin, odd rows -sin.
    """
    freqs = 1.0 / (10000.0 ** (np.arange(0, D, 4, dtype=np.float64) / D))  # [8]
    t = np.arange(G, dtype=np.float64)
    fx = t[:, None] * freqs[None, :]  # [32, 8]
    ax = np.broadcast_to(fx[:, None, :], (G, G, D // 4))
    ay = np.broadcast_to(fx[None, :, :], (G, G, D // 4))
    ang = np.concatenate([ax, ay], axis=-1).reshape(S, D // 2)  # [1024, 16]
    cos = np.cos(ang).astype(np.float32)  # [S, 16]
    sin = np.sin(ang).astype(np.float32)
    p = np.arange(128)
    pair = (p % D) // 2  # [128]
    sgn = np.where(p % 2 == 0, 1.0, -1.0).astype(np.float32)
    cosT = np.tile(cos[:, pair].T, (1, B_LOC))  # [128, T]
    sinT = np.tile(sin[:, pair].T * sgn[:, None], (1, B_LOC))
    return np.ascontiguousarray(cosT), np.ascontiguousarray(sinT)


def _build():
    import concourse.bass as bass  # noqa: F401
    import concourse.tile as tile
    from concourse import bacc, mybir

    f32 = mybir.dt.float32
    # float32r: single-pass fp32 matmul (4x faster than fp32's 2 half-speed
    # passes; TF32-ish multiplier precision). All matmul operands use it.
    f32r = mybir.dt.float32r
    attn_dt = mybir.dt.bfloat16 if ATTN_BF16 else f32

    nc = bacc.Bacc("TRN2", target_bir_lowering=False, debug=False, num_devices=NCORES)

    xT_d = nc.dram_tensor("xT_aug", [E + 1, T], f32, kind="ExternalInput").ap()
    wqk_d = nc.dram_tensor("wqk_aug", [E + 1, 2 * E], f32, kind="ExternalInput").ap()
    wv_d = nc.dram_tensor("wv_aug", [E + 1, E], f32, kind="ExternalInput").ap()
    wo_d = nc.dram_tensor("wo_aug", [E + 1, E], f32, kind="ExternalInput").ap()
    cos_d = nc.dram_tensor("cosT", [128, T], f32, kind="ExternalInput").ap()
    sin_d = nc.dram_tensor("sinT", [128, T], f32, kind="ExternalInput").ap()
    psw_d = nc.dram_tensor("pswap", [128, 128], f32, kind="ExternalInput").ap()
    out_d = nc.dram_tensor("out", [T, E], f32, kind="ExternalOutput").ap()

    with tile.TileContext(nc) as tc:
        consts = tc.alloc_tile_pool(name="consts", bufs=1)
        work = tc.alloc_tile_pool(name="work", bufs=1)

        # ---- constant / weight loads -------------------------------------
        xT_a = consts.tile([128, T], f32, name="xT_a")
        xT_b = consts.tile([128, T], f32, name="xT_b")
        xT_ones = consts.tile([1, T], f32, name="xT_ones")
        nc.sync.dma_start(out=xT_a, in_=xT_d[0:128, :])
        nc.sync.dma_start(out=xT_b, in_=xT_d[128:256, :])
        nc.sync.dma_start(out=xT_ones, in_=xT_d[256:257, :])
        xT_chunks = [xT_a, xT_b, xT_ones]

        wqk_a = consts.tile([128, 2 * E], f32, name="wqk_a")
        wqk_b = consts.tile([128, 2 * E], f32, name="wqk_b")
        wqk_c = consts.tile([1, 2 * E], f32, name="wqk_c")
        nc.sync.dma_start(out=wqk_a, in_=wqk_d[0:128, :])
        nc.sync.dma_start(out=wqk_b, in_=wqk_d[128:256, :])
        nc.sync.dma_start(out=wqk_c, in_=wqk_d[256:257, :])
        wqk_chunks = [wqk_a, wqk_b, wqk_c]

        wv_a = consts.tile([128, E], f32, name="wv_a")
        wv_b = consts.tile([128, E], f32, name="wv_b")
        wv_c = consts.tile([1, E], f32, name="wv_c")
        nc.sync.dma_start(out=wv_a, in_=wv_d[0:128, :])
        nc.sync.dma_start(out=wv_b, in_=wv_d[128:256, :])
        nc.sync.dma_start(out=wv_c, in_=wv_d[256:257, :])
        wv_chunks = [wv_a, wv_b, wv_c]

        wo_a = consts.tile([128, E], f32, name="wo_a")
        wo_b = consts.tile([128, E], f32, name="wo_b")
        wo_c = consts.tile([1, E], f32, name="wo_c")
        nc.sync.dma_start(out=wo_a, in_=wo_d[0:128, :])
        nc.sync.dma_start(out=wo_b, in_=wo_d[128:256, :])
        nc.sync.dma_start(out=wo_c, in_=wo_d[256:257, :])
        wo_chunks = [wo_a, wo_b, wo_c]

        cosT = consts.tile([128, T], f32, name="cosT")
        sinT = consts.tile([128, T], f32, name="sinT")
        pswap = consts.tile([128, 128], f32, name="pswap")
        nc.sync.dma_start(out=cosT, in_=cos_d)
        nc.sync.dma_start(out=sinT, in_=sin_d)
        nc.sync.dma_start(out=pswap, in_=psw_d)

        ones32 = consts.tile([1, 32], f32, name="ones32")
        nc.vector.memset(ones32, 1.0)

        # feature-major roped q/k: 4 chunks of 128 rows (q heads 0-7, k heads 0-7)
        qk_rope = [
            consts.tile([128, T], f32, name=f"qk_rope{m}", tag=f"qk_rope{m}")
            for m in range(4)
        ]
        # v with ones column, token-major: [128, tok_tile, head, 33]
        v_all = consts.tile([128, T // 128, H, D + 1], attn_dt, name="v_all")
        nc.vector.memset(v_all[:, :, :, D : D + 1], 1.0)
        # attention output, feature-major: 2 chunks of 128 rows
        att_oT = [
            consts.tile([128, T], f32, name=f"att_oT{g}", tag=f"att_oT{g}")
            for g in range(2)
        ]

        NSL = T // 512  # 512-col slices over tokens

        # ================= phase 1: qk projection + rope, v projection ====
        with tc.tile_pool(name="ps1", bufs=1, space="PSUM") as ps1:
            for m in range(4):
                qk_ps = ps1.tile([128, T], f32, name=f"qk_ps{m}", tag="qk_ps", bufs=1)
                for n in range(NSL):
                    sl = slice(n * 512, (n + 1) * 512)
                    for k in range(3):
                        nc.tensor.matmul(
                            out=qk_ps[:, sl],
                            lhsT=wqk_chunks[k][:, m * 128 : (m + 1) * 128],
                            rhs=xT_chunks[k][:, sl],
                            start=(k == 0),
                            stop=(k == 2),
                        )
                t_tmp = work.tile([128, T], f32, name="t_tmp", tag="t_tmp", bufs=1)
                nc.vector.tensor_mul(t_tmp, qk_ps, sinT)
                for n in range(NSL):
                    sl = slice(n * 512, (n + 1) * 512)
                    perm_ps = ps1.tile(
                        [128, 512], f32, name="perm_ps", tag="perm_ps", bufs=2
                    )
                    nc.tensor.matmul(
                        out=perm_ps, lhsT=pswap, rhs=t_tmp[:, sl],
                        start=True, stop=True,
                    )
                    nc.vector.tensor_mul(qk_rope[m][:, sl], qk_ps[:, sl], cosT[:, sl])
                    nc.vector.tensor_add(
                        qk_rope[m][:, sl], qk_rope[m][:, sl], perm_ps
                    )

            for tt in range(T // 128):
                tsl = slice(tt * 128, (tt + 1) * 128)
                v_ps = ps1.tile([128, E], f32, name="v_ps", tag="v_ps", bufs=2)
                for k in range(3):
                    lhsT = xT_chunks[k][:, tsl]
                    nc.tensor.matmul(
                        out=v_ps,
                        lhsT=lhsT,
                        rhs=wv_chunks[k],
                        start=(k == 0),
                        stop=(k == 2),
                    )
                nc.vector.tensor_copy(
                    out=v_all[:, tt, :, 0:D], in_=v_ps.rearrange("p (h d) -> p h d", h=H)
                )

        # ================= phase 2: attention ============================
        # ==== phase 3 shares the psum pool (fps shares the bcast slots) ===
        with tc.tile_pool(name="ps2", bufs=1, space="PSUM") as ps2:
            for b in range(B_LOC):
                for g in range(2):  # head group: heads 4g..4g+3
                    qc = qk_rope[g]  # q rows for heads 4g..4g+3
                    kc = qk_rope[2 + g]  # k rows
                    for half in range(2):
                        qsl = slice(b * S + half * 512, b * S + half * 512 + 512)
                        attn_tiles = []
                        for j in range(8):  # sk tile within batch b
                            ksl = slice(b * S + j * 128, b * S + j * 128 + 128)
                            s_ps = ps2.tile(
                                [128, 4, 512], f32, name="s_ps", tag="s_ps", bufs=1
                            )
                            for hh in range(4):
                                psl = slice(32 * hh, 32 * hh + 32)
                                nc.tensor.matmul(
                                    out=s_ps[:, hh, :],
                                    lhsT=kc[psl, ksl],
                                    rhs=qc[psl, qsl],
                                    start=True,
                                    stop=True,
                                    tile_position=(32 * hh, 0),
                                )
                            attn_j = work.tile(
                                [128, 4, 512], attn_dt, name="attn_j", tag="attn",
                                bufs=10,
                            )
                            for eh in range(2):
                                hsl = slice(2 * eh, 2 * eh + 2)
                                nc.scalar.activation(
                                    out=attn_j[:, hsl, :],
                                    in_=s_ps[:, hsl, :],
                                    func=mybir.ActivationFunctionType.Exp,
                                    scale=SCALE,
                                )
                            attn_tiles.append(attn_j)

                        for p in range(2):  # head pair within the group
                            o_ps = ps2.tile(
                                [128, 512], f32, name="o_ps", tag="o_ps", bufs=2
                            )
                            for j in range(8):
                                for w, hh in ((0, 2 * p), (64, 2 * p + 1)):
                                    nc.tensor.matmul(
                                        out=o_ps[w : w + 33, :],
                                        lhsT=v_all[:, b * 8 + j, 4 * g + hh, :],
                                        rhs=attn_tiles[j][:, hh, :],
                                        start=(j == 0),
                                        stop=(j == 7),
                                    )
                            bc_ps = ps2.tile(
                                [128, 512], f32, name="bc_ps", tag="bc_ps", bufs=2
                            )
                            rr = []
                            for w, hh in ((0, 2 * p), (64, 2 * p + 1)):
                                rec = work.tile(
                                    [1, 512], f32, name="recip", tag="recip", bufs=4
                                )
                                with nc.allow_low_precision(
                                    reason="f32r rounding for fast bcast matmul"
                                ):
                                    nc.vector.reciprocal(
                                        out=rec, in_=o_ps[w + 32 : w + 33, :]
                                    )
                                rr.append(rec)
                            for (w, hh), rec in zip(((0, 2 * p), (64, 2 * p + 1)), rr):
                                nc.tensor.matmul(
                                    out=bc_ps[w : w + 32, :],
                                    lhsT=ones32,
                                    rhs=rec,
                                    start=True,
                                    stop=True,
                                )
                            for w, hh in ((0, 2 * p), (64, 2 * p + 1)):
                                o_sb32 = work.tile(
                                    [32, 512], f32, name="o_sb32", tag="o_sb32",
                                    bufs=4,
                                )
                                nc.vector.tensor_copy(
                                    out=o_sb32, in_=o_ps[w : w + 32, :]
                                )
                                nc.vector.tensor_mul(
                                    att_oT[g][32 * hh : 32 * hh + 32, qsl],
                                    o_sb32,
                                    bc_ps[w : w + 32, :],
                                )

            # ============= phase 3: out projection (per batch) ===========
            def out_proj(b):
              for tt in range(b * 8, b * 8 + 8):
                tsl = slice(tt * 128, (tt + 1) * 128)
                f_ps = ps2.tile([128, E], f32, name="f_ps", tag="bc_ps", bufs=2)
                for k in range(3):
                    lhsT = (att_oT[0], att_oT[1], xT_ones)[k][:, tsl]
                    nc.tensor.matmul(
                        out=f_ps,
                        lhsT=lhsT,
                        rhs=wo_chunks[k],
                        start=(k == 0),
                        stop=(k == 2),
                    )
                o_sb = work.tile([128, E], f32, name="o_sb", tag="o_sb", bufs=3)
                nc.vector.tensor_copy(out=o_sb, in_=f_ps)
                nc.sync.dma_start(out=out_d[tsl, :], in_=o_sb)

            for b in range(B_LOC):
                out_proj(b)

        work.release()
        consts.release()

    nc.compile()
    return nc


def _prep_core_inputs(x_loc, Wqkv, bqkv, Wout, bout, cosT, sinT, pswap):
    xT = x_loc.reshape(T, E).T.astype(np.float32)  # [256, T]
    xT_aug = np.ascontiguousarray(
        np.concatenate([xT, np.ones((1, T), np.float32)], axis=0)
    )
    wqk_aug = np.ascontiguousarray(
        np.concatenate([Wqkv[:, : 2 * E], bqkv[None, : 2 * E]], axis=0)
    ).astype(np.float32)
    wv_aug = np.ascontiguousarray(
        np.concatenate([Wqkv[:, 2 * E :], bqkv[None, 2 * E :]], axis=0)
    ).astype(np.float32)
    wo_aug = np.ascontiguousarray(
        np.concatenate([Wout, bout[None, :]], axis=0)
    ).astype(np.float32)
    return {
        "xT_aug": xT_aug,
        "wqk_aug": wqk_aug,
        "wv_aug": wv_aug,
        "wo_aug": wo_aug,
        "cosT": cosT,
        "sinT": sinT,
        "pswap": pswap,
    }


def kernel(x, Wqkv, bqkv, Wout, bout):
    global _COMPILED
    from concourse.bass_utils import run_bass_kernel_spmd

    if _COMPILED is None:
        _COMPILED = _build()
    nc = _COMPILED

    x = np.asarray(x, np.float32)
    Wqkv = np.asarray(Wqkv, np.float32)
    bqkv = np.asarray(bqkv, np.float32)
    Wout = np.asarray(Wout, np.float32)
    bout = np.asarray(bout, np.float32)

    cosT, sinT = _rope_tables()
    pswap = np.zeros((128, 128), np.float32)
    idx = np.arange(128)
    pswap[idx, idx ^ 1] = 1.0

    in_maps = [
        _prep_core_inputs(
            x[c * B_LOC : (c + 1) * B_LOC], Wqkv, bqkv, Wout, bout, cosT, sinT, pswap
        )
        for c in range(NCORES)
    ]
    global LAST_RESULTS
    kw = {}
    if TRACE:
        kw = dict(trace=True, tmpdir=TRACE_DIR)
    res = run_bass_kernel_spmd(nc, in_maps, list(range(NCORES)), **kw)
    LAST_RESULTS = res
    out = np.stack([res.results[c]["out"].reshape(B_LOC, S, E) for c in range(NCORES)])
    return np.ascontiguousarray(out.reshape(B, S, E))

